# revision 51
# baseline (speedup 1.0000x reference)
"""Multi-head self-attention Trainium2 kernel (8-core data-parallel over batch).

Problem: B=8, N=1024, D=768, H=12, Hd=64 MHSA with qkv/proj projections.
Sharding: batch-parallel, one batch item per NeuronCore; no collectives.

Per-core dataflow (all 2-byte tensors are fp16: same PE/DVE/DMA cost as
bf16 but 8x finer mantissa; matmuls accumulate fp32 in PSUM):
  x --SWDGE cast-DMA (8 chunks)--> fp16 --PE transpose--> xT
  Q^T/K^T = Wqk^T X^T per head pair (flat weight tile, 512B DMA runs),
      per-partition bias on DVE -> qkt
  V = X Wv + bcast(bv) (bias rides V: softmax rows sum to 1, so V+bv
      shifts the normalized output by exactly bv; bcast built once on
      GpSimd partition_broadcast)
  S^T[j,i] = K^T.T Q^T  (K=64 strips, head pair packed at partitions 0/64)
  P = exp(S^T/8)  (ScalarE, scale folded into the activation; fp16 out)
  [O'^T; r] = [V|1]^T P accumulated over j-tiles (M=65, r = softmax denom)
  normalize: r row -> partition 0 (DMA) -> partition_broadcast (GpSimd) ->
      reciprocal (DVE) -> O' * (1/r) (DVE); odd head reaches oT's upper
      partitions via DMA
  out = O Wproj + bcast(bp)  (bias added during the PSUM->SBUF eviction)

Schedule: ScalarE's exp stream (96 x [128,1024], ~1.04us each) is the
phase-2 bottleneck, so pair 0+1's scores/exps are emitted during phase 1
(banked in a 28-deep fp16 pool) and phase-2 scores run at 6-per-8 slots
(two scoreless slots per pair at jt 3/7) so the sp-PSUM recycling stays
ahead of the PE. At jt==7 the even head's eviction is emitted between the
two heads' matmul blocks so the next pair's first AV never WAR-waits.
Phase 3 runs the projection as PSUM pieces: tiles 0/1 (full-width, in
freed scores-PSUM slots) plus 384-col 1-bank half-pieces, with pk0-4
front-chunks covering the pair-5 normalize chain and pk=5 folded first
into the accumulation for late tiles; bias-adds + stores are spread
across sync/scalar queues piece by piece.
"""
import numpy as np
from contextlib import ExitStack

import concourse.bacc as bacc
import concourse.bass as bass
import concourse.tile as tile
from concourse import mybir
from concourse.bass_utils import run_bass_kernel_spmd
from concourse.masks import make_identity

F32 = mybir.dt.float32
# fp16 everywhere a 2-byte dtype is used: same PE/DVE/DMA cost as bf16 but
# 8x finer mantissa (all tensors here are well inside fp16 range)
BF16 = mybir.dt.float16
AF = mybir.ActivationFunctionType

P = 128
SEQ = 1024
D = 768
H = 12
HD = 64
NT = SEQ // P   # 8 seq tiles
KT = D // P     # 6 embed tiles
NPAIR = H // 2  # 6 head pairs
SCALE = 1.0 / np.sqrt(HD)  # folded into exp


def build_mhsa(nc: bass.Bass):
    x = nc.dram_tensor("x", [SEQ, D], F32, kind="ExternalInput").ap()
    qkv_w = nc.dram_tensor("qkv_w", [D, 3 * D], F32, kind="ExternalInput").ap()
    qkv_b = nc.dram_tensor("qkv_b", [3 * D], F32, kind="ExternalInput").ap()
    proj_w = nc.dram_tensor("proj_w", [D, D], F32, kind="ExternalInput").ap()
    proj_b = nc.dram_tensor("proj_b", [D], F32, kind="ExternalInput").ap()
    out = nc.dram_tensor("out", [SEQ, D], F32, kind="ExternalOutput").ap()

    # qkv weight view: p k a (c d) with (c d) = 6 head-pairs x 128 cols kept
    # contiguous, so each half-load moves 768B+ runs (big DMA descriptors)
    qkv_c = qkv_w.rearrange("(k p) (a e) -> p k a e", p=P, a=3)

    with tile.TileContext(nc) as tc, ExitStack() as ctx:
        const = ctx.enter_context(tc.tile_pool(name="const", bufs=1))
        persist = ctx.enter_context(tc.tile_pool(name="persist", bufs=1))

        # ---- persistent arrays ----
        # Q^T/K^T per pair (heads 2p/2p+1 at partitions 0:64/64:128)
        qkt = [
            (
                persist.tile([P, SEQ], BF16, tag=f"qt{p}", name=f"qt{p}"),
                persist.tile([P, SEQ], BF16, tag=f"kt{p}", name=f"kt{p}"),
            )
            for p in range(NPAIR)
        ]
        # V rows: per head h columns [65h:65h+64] = V, column 65h+64 = ones
        vr = [
            persist.tile([P, 65 * H], BF16, tag=f"vr{t}", name=f"vr{t}")
            for t in range(NT)
        ]
        # O^T head-pair tiles (normalized): heads 2k/2k+1 at partitions 0/64
        oT = [
            persist.tile([P, SEQ], BF16, tag=f"oT{k}", name=f"oT{k}")
            for k in range(KT)
        ]
        bias_bc = persist.tile([P, D], BF16, tag="bias_bc")

        xT_pool = ctx.enter_context(tc.tile_pool(name="xTp", bufs=1))
        xT = [
            xT_pool.tile([P, SEQ], BF16, tag=f"xT{k}", name=f"xT{k}")
            for k in range(KT)
        ]

        # normalization scratch (long-lived)
        r0_pool = ctx.enter_context(tc.tile_pool(name="r0", bufs=2))
        rbc_pool = ctx.enter_context(tc.tile_pool(name="rbc", bufs=3))
        rec_pool = ctx.enter_context(tc.tile_pool(name="rec", bufs=3))
        otb_pool = ctx.enter_context(tc.tile_pool(name="otb", bufs=2))
        osb_pool = ctx.enter_context(tc.tile_pool(name="osb", bufs=3))

        # ================= phase 1 =================
        # ---- input DMAs (device order ~ issue order: x first), then
        # x -> xT (PE transposes with bf16 identity: 1 cyc/row) ----
        # All big operands are bf16 (the HW verifier forbids mixing f32r with
        # bf16 in a matmul, and bf16 halves DMA bytes at the same PE rate).
        # x is cast to bf16 during the SWDGE load so the transposes run at
        # 1 cyc/row. Pool-queue order: x chunks + pair-0/1 QK weights + bqk
        # first (they gate the first PE work), then the rest.
        # QK weights land in one flat tile, loaded in 2-pair chunks so each
        # DMA run is 512B contiguous (the per-pair layout had 256B runs ->
        # 2x descriptor latency in the DGE) while pair 0/1 still arrives
        # right after x.
        wqk_t = const.tile([P, KT, 2, NPAIR * P], BF16, tag="wqk")

        def emit_wqk_load(a, chunk):
            lo = chunk * 2 * P
            nc.gpsimd.dma_start(
                wqk_t[:, :, a, lo : lo + 2 * P],
                qkv_c[:, :, a, lo : lo + 2 * P],
            )

        # constants first: the identity gates the very first transposes, so
        # it must not sit behind the x DMA on the Pool queue
        ident = const.tile([P, P], BF16, tag="ident")
        make_identity(nc, ident)
        ones_st = const.tile([P, P], F32, tag="ones_st")
        nc.gpsimd.memset(ones_st[:], 1.0)
        ones_h = const.tile([P, H], BF16, tag="ones_h")
        nc.vector.tensor_copy(ones_h[:], ones_st[:, 0:H])

        # x staging: one big tile, chunked SWDGE cast-DMAs; fine first chunks
        # so the first transposes (and thus the first QK matmuls) start ASAP
        xst_ctx = ExitStack()
        xst_pool = xst_ctx.enter_context(tc.tile_pool(name="xst", bufs=1))
        xn = xst_pool.tile([P, NT, D], BF16, tag="xn", name="xn")
        xr = x.rearrange("(g p) d -> p g d", p=P)
        nc.gpsimd.dma_start(xn[:, 0:1, :], xr[:, 0:1, :])
        nc.gpsimd.dma_start(xn[:, 1:2, :], xr[:, 1:2, :])

        if True:
            nc.gpsimd.dma_start(xn[:, 2:4, :], xr[:, 2:4, :])
            emit_wqk_load(0, 0)
            nc.gpsimd.dma_start(xn[:, 4:6, :], xr[:, 4:6, :])
            emit_wqk_load(1, 0)
            nc.gpsimd.dma_start(xn[:, 6:8, :], xr[:, 6:8, :])
            bqk = const.tile([P, H], F32, tag="bqk")
            nc.gpsimd.dma_start(
                bqk[:], qkv_b[0 : H * P].rearrange("(t p) -> p t", p=P)
            )
            wv = const.tile([P, KT, D], BF16, tag="wv")
            nc.gpsimd.dma_start(
                wv[:], qkv_w[:, 2 * D : 3 * D].rearrange("(k p) d -> p k d", p=P)
            )
            for chunk in (1, 2):
                emit_wqk_load(0, chunk)
                emit_wqk_load(1, chunk)
            # bias rows (fp16 cast on load); broadcast across partitions on
            # the (otherwise idle) GpSimd engine. bv rides V (softmax rows
            # sum to 1 so adding bv to V adds bv to the normalized output);
            # bp is added during the proj eviction. This replaces the PE
            # bias-build matmul chain.
            bv_row = const.tile([1, D], BF16, tag="bv_row")
            nc.gpsimd.dma_start(
                bv_row[:], qkv_b[2 * D : 3 * D].rearrange("(o d) -> o d", o=1)
            )
            wp = const.tile([P, KT, D], BF16, tag="wp")
            nc.gpsimd.dma_start(wp[:], proj_w.rearrange("(k p) d -> p k d", p=P))
            bp_row = const.tile([1, D], BF16, tag="bp_row")
            nc.gpsimd.dma_start(bp_row[:], proj_b.rearrange("(o d) -> o d", o=1))
            bv_bc = const.tile([P, D], BF16, tag="bv_bc")
            nc.gpsimd.partition_broadcast(bv_bc[:], bv_row[:], channels=P)
            nc.gpsimd.partition_broadcast(bias_bc[:], bp_row[:], channels=P)
            # upper half of the last proj-weight chunk shifted to partitions
            # 0:64 (pairs with the odd head's pre-DMA normalize output in the
            # split pk=5 projection matmul)
            wp5hi = const.tile([HD, D], BF16, tag="wp5hi")
            nc.sync.dma_start(wp5hi[:], wp[HD:P, KT - 1, :])

            with tc.tile_pool(name="xt_ps", bufs=4, space="PSUM") as xtps_pool:
                for g in range(NT // 2):
                    for k in range(KT):
                        tp = xtps_pool.tile([P, 256], BF16, name="xtp")
                        for q in range(2):
                            nc.tensor.transpose(
                                tp[:, q * P : (q + 1) * P],
                                xn[:, 2 * g + q, k * P : (k + 1) * P],
                                ident[:],
                            )
                        if k < 3:
                            nc.vector.tensor_copy(
                                xT[k][:, g * 256 : (g + 1) * 256], tp[:]
                            )
                        else:
                            nc.scalar.activation(
                                xT[k][:, g * 256 : (g + 1) * 256], tp[:], AF.Copy
                            )
            xst_ctx.close()

        pexp_pool = ctx.enter_context(tc.tile_pool(name="pexp", bufs=28))
        sp_ctx = ExitStack()
        sp_pool = sp_ctx.enter_context(tc.tile_pool(name="s_ps", bufs=2, space="PSUM"))

        pe_store = {}  # (pair, jt) -> (peA, peB)

        def emit_scores(pr, jt):
            qt, kt = qkt[pr]
            sps = []
            for plo in (0, HD):
                sp = sp_pool.tile([P, SEQ], F32, tag="sp", name="sp")
                for lo in (0, 512):
                    nc.tensor.matmul(
                        sp[:, lo : lo + 512],
                        kt[plo : plo + HD, jt * P : (jt + 1) * P],
                        qt[plo : plo + HD, lo : lo + 512],
                        start=True, stop=True,
                        tile_position=(plo, 0),
                    )
                sps.append(sp)
            pes = []
            for sp in sps:
                pe = pexp_pool.tile([P, SEQ], BF16, tag="pe", name="pe")
                nc.scalar.activation(pe[:], sp[:], AF.Exp, bias=0.0, scale=float(SCALE))
                pes.append(pe)
            pe_store[(pr, jt)] = pes

        def emit_qk_tile(pr, a, qps_pool):
            # a=0 -> Q^T, a=1 -> K^T of pair pr
            qp = qps_pool.tile([P, SEQ], F32, tag="qp", name="qp")
            for lo in (0, 512):
                for k in range(KT):
                    nc.tensor.matmul(
                        qp[:, lo : lo + 512],
                        wqk_t[:, k, a, pr * P : (pr + 1) * P],
                        xT[k][:, lo : lo + 512],
                        start=(k == 0), stop=(k == KT - 1),
                    )
            qe = qkt[pr][a]
            with nc.allow_low_precision(reason="bf16 qkt"):
                nc.vector.tensor_scalar_add(
                    qe[:], qp[:], bqk[:, (a * NPAIR + pr) : (a * NPAIR + pr) + 1]
                )

        # ---- QK pairs 0-1 (enables early exp start on ScalarE) ----
        with tc.tile_pool(name="q_ps", bufs=2, space="PSUM") as qps_pool:
            for pr in (0, 1):
                for a in (0, 1):
                    emit_qk_tile(pr, a, qps_pool)

        # ---- V (interleaved with pair-0 scores+exp) ----
        def emit_v(nt):
            vp = vps_pool.tile([P, D], F32, name="vp")
            for lo, sz in ((0, 512), (512, 256)):
                for k in range(KT):
                    nc.tensor.matmul(
                        vp[:, lo : lo + sz],
                        xT[k][:, nt * P : (nt + 1) * P],
                        wv[:, k, lo : lo + sz],
                        start=(k == 0), stop=(k == KT - 1),
                    )
            v3 = vr[nt].rearrange("p (h c) -> p h c", h=H, c=65)
            nc.vector.tensor_copy(
                v3[:, :, 64:65], ones_h.rearrange("p (h o) -> p h o", h=H, o=1)
            )
            # on DVE (not ScalarE): keeps the activation engine free so the
            # pre-computed exps start as early as possible. The V bias is
            # added here (rows of softmax sum to 1, so V+bv shifts the
            # normalized output by exactly bv).
            with nc.allow_low_precision(reason="fp16 V"):
                nc.vector.tensor_add(
                    v3[:, :, 0:64],
                    vp.rearrange("p (h c) -> p h c", h=H, c=HD),
                    bv_bc.rearrange("p (h c) -> p h c", h=H, c=HD),
                )

        with tc.tile_pool(name="v_ps", bufs=2, space="PSUM") as vps_pool:
            for nt in range(NT):
                emit_v(nt)
                emit_scores(0, nt)

        # ---- QK pairs 2-5 (interleaved with pair-1 scores jt 0-3) ----
        with tc.tile_pool(name="q_ps2", bufs=2, space="PSUM") as qps_pool:
            units = [(pr, a) for pr in range(2, NPAIR) for a in (0, 1)]
            for i, (pr, a) in enumerate(units):
                emit_qk_tile(pr, a, qps_pool)
                if i < 6:
                    emit_scores(1, i)
            emit_scores(1, 6)
            emit_scores(1, 7)

        # ================= phase 2: attention =================
        def emit_av(pr, jt, oA, oB, evict=False):
            # evict=True (jt==NT-1, mid pairs): emit oA's eviction between
            # the two heads' matmul blocks so it overlaps oB's AV + the
            # boundary scores -- the next pair's first AV WAR-waits on it
            peA, peB = pe_store.pop((pr, jt))
            evs = []
            for h, o_ps, pe in ((2 * pr, oA, peA), (2 * pr + 1, oB, peB)):
                for lo in (0, 512):
                    nc.tensor.matmul(
                        o_ps[:, lo : lo + 512],
                        vr[jt][:, 65 * h : 65 * h + 65],
                        pe[:, lo : lo + 512],
                        start=(jt == 0), stop=(jt == NT - 1),
                    )
                if evict:
                    evs.append(emit_evict(o_ps))
            return tuple(evs) if evict else None

        def emit_evict(o_ps, on_act=False):
            osb = osb_pool.tile([65, SEQ], BF16, tag="osb", name="osb")
            with nc.allow_low_precision(reason="O' partial sums bf16"):
                if on_act:
                    nc.scalar.activation(osb[:], o_ps[:], AF.Copy)
                else:
                    nc.vector.tensor_copy(osb[:], o_ps[:])
            return osb

        def emit_normalize(pr, oAs, oBs, b_first=False, skip_b_dma=False):
            # per head: r (row 64) -> bcast over 64 partitions (GpSimd),
            # 1/r (DVE), O'*1/r (DVE). Odd head lands at partitions 64:128
            # of oT via DMA (DVE lanes can't shift partitions).
            # r row (partition 64) -> partition 0 via DMA; the broadcast
            # ucode only reads partition 0
            ret = None
            order = (1, 0) if b_first else (0, 1)
            for side in order:
                osrc = (oAs, oBs)[side]
                r0 = r0_pool.tile([1, SEQ], BF16, tag="r0", name="r0")
                nc.sync.dma_start(r0[:], osrc[64:65, :])
                rbc = rbc_pool.tile([HD, SEQ], BF16, tag="rbc", name="rbc")
                nc.gpsimd.partition_broadcast(rbc[:], r0[:], channels=HD)
                rec = rec_pool.tile([HD, SEQ], BF16, tag="rec", name="rec")
                with nc.allow_low_precision(reason="softmax denom recip"):
                    nc.vector.reciprocal(rec[:], rbc[:])
                with nc.allow_low_precision(reason="bf16 oT"):
                    if side == 0:
                        nc.vector.tensor_mul(
                            oT[pr][0:HD, :], osrc[0:HD, :], rec[:]
                        )
                    else:
                        ot_b = otb_pool.tile([HD, SEQ], BF16, tag="otb", name="otb")
                        nc.vector.tensor_mul(ot_b[:], osrc[0:HD, :], rec[:])
                        if not skip_b_dma:
                            nc.sync.dma_start(oT[pr][HD:P, :], ot_b[:])
                        ret = ot_b
            return ret

        def emit_normalize_last(pr, osbA, osbB):
            # pair-5 tail: both evictions run on ACT, so the DVE pipelines
            # recipA -> mulA with minimum latency; the even head's
            # oT[pr][0:64] is what gates the first pk=5 projection matmuls.
            r0A = r0_pool.tile([1, SEQ], BF16, tag="r0", name="r0A")
            nc.sync.dma_start(r0A[:], osbA[64:65, :])
            r0B = r0_pool.tile([1, SEQ], BF16, tag="r0", name="r0B")
            nc.sync.dma_start(r0B[:], osbB[64:65, :])
            rbcA = rbc_pool.tile([HD, SEQ], BF16, tag="rbc", name="rbcA")
            nc.gpsimd.partition_broadcast(rbcA[:], r0A[:], channels=HD)
            rbcB = rbc_pool.tile([HD, SEQ], BF16, tag="rbc", name="rbcB")
            nc.gpsimd.partition_broadcast(rbcB[:], r0B[:], channels=HD)
            recA = rec_pool.tile([HD, SEQ], BF16, tag="rec", name="recA")
            recB = rec_pool.tile([HD, SEQ], BF16, tag="rec", name="recB")
            with nc.allow_low_precision(reason="softmax denom recip"):
                nc.vector.reciprocal(recA[:], rbcA[:])
            with nc.allow_low_precision(reason="bf16 oT"):
                nc.vector.tensor_mul(oT[pr][0:HD, :], osbA[0:HD, :], recA[:])
            with nc.allow_low_precision(reason="softmax denom recip"):
                nc.vector.reciprocal(recB[:], rbcB[:])
            ot_b = otb_pool.tile([HD, SEQ], BF16, tag="otb", name="otb")
            with nc.allow_low_precision(reason="bf16 oT"):
                nc.vector.tensor_mul(ot_b[:], osbB[0:HD, :], recB[:])
            return ot_b

        # ---- proj helpers (defined early: the first two proj front-chunks
        # are emitted inside the o-pool block, before it closes) ----
        pps = {}
        otb5_box = []
        ost_ctx = ExitStack()
        ost_pool = ost_ctx.enter_context(tc.tile_pool(name="ost", bufs=4))
        osth_pool = ost_ctx.enter_context(tc.tile_pool(name="osth", bufs=6))

        def emit_proj_chunks(pool, it, pks, start, stop, half=None):
            # half=None: full-width tile in a freed scores-PSUM slot (2
            # banks); half=0/1: a 384-col 1-bank piece -- fine-grained slot
            # recycling so late tiles never wait on a bias-add to free PSUM
            key = (it, half)
            if key not in pps:
                if half is None:
                    t = pool.tile([P, SEQ], F32, tag="sp", name="pp_sp")
                    pps[key] = t[:, 0:D]
                else:
                    pps[key] = pool.tile([P, 384], F32, tag="pph", name="pph")
            pp = pps[key]
            base = 0 if half is None else half * 384
            segs = ((0, 512), (512, 256)) if half is None else ((0, 384),)
            for lo, sz in segs:
                gl = base + lo
                for i, pk in enumerate(pks):
                    last = stop and i == len(pks) - 1
                    if pk == NPAIR - 1:
                        nc.tensor.matmul(
                            pp[:, lo : lo + sz],
                            oT[pk][0:HD, it * P : (it + 1) * P],
                            wp[0:HD, pk, gl : gl + sz],
                            start=(start and i == 0), stop=False,
                            tile_position=(0, 0),
                        )
                        nc.tensor.matmul(
                            pp[:, lo : lo + sz],
                            otb5_box[0][:, it * P : (it + 1) * P],
                            wp5hi[:, gl : gl + sz],
                            start=False, stop=last,
                            tile_position=(0, 0),
                        )
                    else:
                        nc.tensor.matmul(
                            pp[:, lo : lo + sz],
                            oT[pk][:, it * P : (it + 1) * P],
                            wp[:, pk, gl : gl + sz],
                            start=(start and i == 0), stop=last,
                        )

        # Single software-pipelined stream: the score/exp stream runs ~12
        # slots ahead of the AV stream (pair 0 + pair 1 jt0-3 banked during
        # phase 1), so ScalarE (the P2-bottleneck engine) never starves and
        # the PE never waits on exp.
        pending = {}  # pair -> (oAs, oBs)
        s_stream = [
            (p, jt) for p in range(2, NPAIR) for jt in range(NT)
        ]
        si = 0
        with tc.tile_pool(name="o_ps", bufs=2, space="PSUM") as ops_pool:
            for p in range(NPAIR):
                oA = ops_pool.tile([65, SEQ], F32, tag="o", name="oA")
                oB = ops_pool.tile([65, SEQ], F32, tag="o", name="oB")
                last_mid = p < NPAIR - 1
                for jt in range(NT):
                    # scores at 6-per-8 slots: the exp stream (2 exps per
                    # emission, ~2.08us) outpaces a 1/slot PE cadence
                    # (~1.71us); two scoreless slots per pair keep the
                    # sp-psum recycling ahead of the PE. They sit at jt 3/7
                    # so the jt==7 slot (which also carries the eviction
                    # interleave) never stalls on the ACT stream.
                    if si < len(s_stream) and jt not in (3, NT - 1):
                        emit_scores(*s_stream[si])
                        si += 1
                    evs = emit_av(
                        p, jt, oA, oB, evict=(last_mid and jt == NT - 1)
                    )
                    if jt == 0 and (p - 1) in pending:
                        emit_normalize(p - 1, *pending.pop(p - 1))
                if p == NPAIR - 1:
                    # A-side eviction on the (now idle) scalar engine, B on
                    # DVE: they run in parallel and the o-psum banks free
                    # for the first pp-pool proj pieces ~1us sooner
                    evA = emit_evict(oA, on_act=True)
                    evB = emit_evict(oB)
                else:
                    pending[p] = evs

            # ================= phase 3: proj =================
            # last pair's normalization overlaps the first proj chunks; its
            # oT contribution (pk=5) is ordered last in every accumulation
            # and, for the odd head, is taken straight from the pre-DMA
            # normalize output (ot_b at partitions 0:64) as a second K=64
            # matmul so the partition-shifting DMA never gates the proj.
            # f0/f1 ride freed scores-PSUM slots and are emitted before the
            # o-pool closes: they keep the PE fed across the pool-close
            # barrier (which waits on the pair-5 evictions)
            emit_proj_chunks(sp_pool, 0, range(5), start=True, stop=False)
            emit_proj_chunks(sp_pool, 1, range(5), start=True, stop=False)

        # pair-5 normalize AFTER the o-pool closes: its DVE ops would
        # otherwise sit ahead of the pool-release drain on the DVE queue and
        # delay the first pp-pool proj pieces by the whole chain
        otb5_box.append(emit_normalize_last(NPAIR - 1, evA, evB))

        def emit_proj_out(it, q, half=None):
            pp = pps.pop((it, half))
            if half is None:
                ot = ost_pool.tile([P, D], F32, tag="ot", name="ot")
                for (lo, sz), sq in zip(((0, 384), (384, 384)),
                                        (nc.sync, nc.scalar)):
                    nc.vector.tensor_add(
                        ot[:, lo : lo + sz], pp[:, lo : lo + sz],
                        bias_bc[:, lo : lo + sz],
                    )
                    sq.dma_start(
                        out[it * P : (it + 1) * P, lo : lo + sz],
                        ot[:, lo : lo + sz],
                    )
            else:
                base = half * 384
                ot = osth_pool.tile([P, 384], F32, tag="oth", name="oth")
                nc.vector.tensor_add(ot[:], pp[:], bias_bc[:, base : base + 384])
                q.dma_start(
                    out[it * P : (it + 1) * P, base : base + 384], ot[:]
                )

        with tc.tile_pool(name="p_ps", bufs=4, space="PSUM") as pp_pool:
            qs = [nc.sync, nc.scalar]
            # tiles 0/1 (full, sp slots, emitted in-pool) + tiles 2/3 as
            # 1-bank half-pieces cover the pair-5 normalize chain; their pk5
            # tails run once it lands. Tiles 4-7 are emitted after the chain
            # completes, so they run as one g-chunk with pk5 FIRST (start)
            # and pk0-4 behind (stop) -- no pk5 stall on the PE queue head.
            # Half-pieces recycle PSUM at 1-bank granularity so the PE never
            # waits on a bias-add.
            seq = [
                ("f", 2, 0), ("f", 2, 1), ("f", 3, 0), ("f", 3, 1),
                ("t", 0, None), ("t", 1, None),
                ("g", 4, None), ("g", 5, None),
                ("t", 2, 0), ("t", 2, 1), ("t", 3, 0), ("t", 3, 1),
                ("g", 6, 0), ("g", 6, 1), ("g", 7, 0), ("g", 7, 1),
            ]
            for n, (kind, it, half) in enumerate(seq):
                pool = sp_pool if half is None else pp_pool
                if kind == "f":
                    # scheduler hint: these pieces are pure gap-fillers for
                    # the pair-5 normalize window -- keep them ahead of the
                    # pk5-dependent chunks on the PE queue
                    with tc.high_priority():
                        emit_proj_chunks(pool, it, range(5), start=True,
                                         stop=False, half=half)
                elif False:
                    pass
                else:
                    pks = [5, 0, 1, 2, 3, 4] if kind == "g" else [5]
                    emit_proj_chunks(pool, it, pks, start=(kind == "g"),
                                     stop=True, half=half)
                    emit_proj_out(it, qs[n % 2], half=half)
        sp_ctx.close()
        ost_ctx.close()

    return nc


_NC_CACHE = {}


def _get_nc():
    if "nc" not in _NC_CACHE:
        nc = bacc.Bacc("TRN2", target_bir_lowering=False, debug=False)
        build_mhsa(nc)
        nc.compile()
        _NC_CACHE["nc"] = nc
    return _NC_CACHE["nc"]


def kernel(x, qkv_w, qkv_b, proj_w, proj_b, _trace=False, _trace_kwargs=None):
    x = np.ascontiguousarray(np.asarray(x, dtype=np.float32))
    B = x.shape[0]
    assert x.shape == (B, SEQ, D)
    nc = _get_nc()
    shared = {
        "qkv_w": np.ascontiguousarray(np.asarray(qkv_w, np.float32)),
        "qkv_b": np.ascontiguousarray(np.asarray(qkv_b, np.float32)),
        "proj_w": np.ascontiguousarray(np.asarray(proj_w, np.float32)),
        "proj_b": np.ascontiguousarray(np.asarray(proj_b, np.float32)),
    }
    in_maps = [{"x": x[b], **shared} for b in range(B)]
    res = run_bass_kernel_spmd(
        nc, in_maps, list(range(B)), trace=_trace, **(_trace_kwargs or {})
    )
    out = np.stack([res.results[b]["out"] for b in range(B)])
    if _trace:
        return out, res
    return out



# revision 60
# speedup vs baseline: 1.0074x; 1.0074x over previous
"""Multi-head self-attention Trainium2 kernel (8-core data-parallel over batch).

Problem: B=8, N=1024, D=768, H=12, Hd=64 MHSA with qkv/proj projections.
Sharding: batch-parallel, one batch item per NeuronCore; no collectives.

Per-core dataflow (all 2-byte tensors are fp16: same PE/DVE/DMA cost as
bf16 but 8x finer mantissa; matmuls accumulate fp32 in PSUM):
  x --SWDGE cast-DMA (8 chunks)--> fp16 --PE transpose--> xT
  Q^T/K^T = Wqk^T X^T per head pair (flat weight tile, 512B DMA runs),
      per-partition bias on DVE -> qkt
  V = X Wv + bcast(bv) (bias rides V: softmax rows sum to 1, so V+bv
      shifts the normalized output by exactly bv; bcast built once on
      GpSimd partition_broadcast)
  S^T[j,i] = K^T.T Q^T  (K=64 strips, head pair packed at partitions 0/64)
  P = exp(S^T/8)  (ScalarE, scale folded into the activation; fp16 out)
  [O'^T; r] = [V|1]^T P accumulated over j-tiles (M=65, r = softmax denom)
  normalize: r row -> partition 0 (DMA) -> partition_broadcast (GpSimd) ->
      reciprocal (DVE) -> O' * (1/r) (DVE); odd head reaches oT's upper
      partitions via DMA
  out = O Wproj + bcast(bp)  (bias added during the PSUM->SBUF eviction)

Schedule: ScalarE's exp stream (96 x [128,1024], ~1.04us each) is the
phase-2 bottleneck, so pair 0+1's scores/exps are emitted during phase 1
(banked in a 28-deep fp16 pool) and phase-2 scores run at 6-per-8 slots
(two scoreless slots per pair at jt 3/7) so the sp-PSUM recycling stays
ahead of the PE. At jt==7 the even head's eviction is emitted between the
two heads' matmul blocks so the next pair's first AV never WAR-waits.
Phase 3 runs the projection as PSUM pieces: tiles 0/1 (full-width, in
freed scores-PSUM slots) plus 384-col 1-bank half-pieces, with pk0-4
front-chunks covering the pair-5 normalize chain and pk=5 folded first
into the accumulation for late tiles; bias-adds + stores are spread
across sync/scalar queues piece by piece.
"""
import numpy as np
from contextlib import ExitStack

import concourse.bacc as bacc
import concourse.bass as bass
import concourse.tile as tile
from concourse import mybir
from concourse.bass_utils import run_bass_kernel_spmd
from concourse.masks import make_identity

F32 = mybir.dt.float32
# fp16 everywhere a 2-byte dtype is used: same PE/DVE/DMA cost as bf16 but
# 8x finer mantissa (all tensors here are well inside fp16 range)
BF16 = mybir.dt.float16
AF = mybir.ActivationFunctionType

P = 128
SEQ = 1024
D = 768
H = 12
HD = 64
NT = SEQ // P   # 8 seq tiles
KT = D // P     # 6 embed tiles
NPAIR = H // 2  # 6 head pairs
SCALE = 1.0 / np.sqrt(HD)  # folded into exp


def build_mhsa(nc: bass.Bass):
    x = nc.dram_tensor("x", [SEQ, D], F32, kind="ExternalInput").ap()
    qkv_w = nc.dram_tensor("qkv_w", [D, 3 * D], F32, kind="ExternalInput").ap()
    qkv_b = nc.dram_tensor("qkv_b", [3 * D], F32, kind="ExternalInput").ap()
    proj_w = nc.dram_tensor("proj_w", [D, D], F32, kind="ExternalInput").ap()
    proj_b = nc.dram_tensor("proj_b", [D], F32, kind="ExternalInput").ap()
    out = nc.dram_tensor("out", [SEQ, D], F32, kind="ExternalOutput").ap()

    # qkv weight view: p k a (c d) with (c d) = 6 head-pairs x 128 cols kept
    # contiguous, so each half-load moves 768B+ runs (big DMA descriptors)
    qkv_c = qkv_w.rearrange("(k p) (a e) -> p k a e", p=P, a=3)

    with tile.TileContext(nc) as tc, ExitStack() as ctx:
        const = ctx.enter_context(tc.tile_pool(name="const", bufs=1))
        persist = ctx.enter_context(tc.tile_pool(name="persist", bufs=1))

        # ---- persistent arrays ----
        # Q^T/K^T per pair (heads 2p/2p+1 at partitions 0:64/64:128)
        qkt = [
            (
                persist.tile([P, SEQ], BF16, tag=f"qt{p}", name=f"qt{p}"),
                persist.tile([P, SEQ], BF16, tag=f"kt{p}", name=f"kt{p}"),
            )
            for p in range(NPAIR)
        ]
        # V rows: per head h columns [65h:65h+64] = V, column 65h+64 = ones
        vr = [
            persist.tile([P, 65 * H], BF16, tag=f"vr{t}", name=f"vr{t}")
            for t in range(NT)
        ]
        # O^T head-pair tiles (normalized): heads 2k/2k+1 at partitions 0/64
        oT = [
            persist.tile([P, SEQ], BF16, tag=f"oT{k}", name=f"oT{k}")
            for k in range(KT)
        ]
        bias_bc = persist.tile([P, D], BF16, tag="bias_bc")

        xT_pool = ctx.enter_context(tc.tile_pool(name="xTp", bufs=1))
        xT = [
            xT_pool.tile([P, SEQ], BF16, tag=f"xT{k}", name=f"xT{k}")
            for k in range(KT)
        ]

        # normalization scratch (long-lived)
        r0_pool = ctx.enter_context(tc.tile_pool(name="r0", bufs=2))
        rbc_pool = ctx.enter_context(tc.tile_pool(name="rbc", bufs=2))
        rec_pool = ctx.enter_context(tc.tile_pool(name="rec", bufs=2))
        otb_pool = ctx.enter_context(tc.tile_pool(name="otb", bufs=2))
        osb_pool = ctx.enter_context(tc.tile_pool(name="osb", bufs=3))

        # ================= phase 1 =================
        # ---- input DMAs (device order ~ issue order: x first), then
        # x -> xT (PE transposes with bf16 identity: 1 cyc/row) ----
        # All big operands are bf16 (the HW verifier forbids mixing f32r with
        # bf16 in a matmul, and bf16 halves DMA bytes at the same PE rate).
        # x is cast to bf16 during the SWDGE load so the transposes run at
        # 1 cyc/row. Pool-queue order: x chunks + pair-0/1 QK weights + bqk
        # first (they gate the first PE work), then the rest.
        # QK weights land in one flat tile, loaded in 2-pair chunks so each
        # DMA run is 512B contiguous (the per-pair layout had 256B runs ->
        # 2x descriptor latency in the DGE) while pair 0/1 still arrives
        # right after x.
        wqk_t = const.tile([P, KT, 2, NPAIR * P], BF16, tag="wqk")

        def emit_wqk_load(a, chunk):
            lo = chunk * 2 * P
            nc.gpsimd.dma_start(
                wqk_t[:, :, a, lo : lo + 2 * P],
                qkv_c[:, :, a, lo : lo + 2 * P],
            )

        # constants first: the identity gates the very first transposes, so
        # it must not sit behind the x DMA on the Pool queue
        ident = const.tile([P, P], BF16, tag="ident")
        make_identity(nc, ident)
        ones_st = const.tile([P, P], F32, tag="ones_st")
        nc.gpsimd.memset(ones_st[:], 1.0)
        ones_h = const.tile([P, H], BF16, tag="ones_h")
        nc.vector.tensor_copy(ones_h[:], ones_st[:, 0:H])

        # x staging: one big tile, chunked SWDGE cast-DMAs; fine first chunks
        # so the first transposes (and thus the first QK matmuls) start ASAP
        xst_ctx = ExitStack()
        xst_pool = xst_ctx.enter_context(tc.tile_pool(name="xst", bufs=1))
        xn = xst_pool.tile([P, NT, D], BF16, tag="xn", name="xn")
        xr = x.rearrange("(g p) d -> p g d", p=P)
        nc.gpsimd.dma_start(xn[:, 0:1, :], xr[:, 0:1, :])
        nc.gpsimd.dma_start(xn[:, 1:2, :], xr[:, 1:2, :])

        if True:
            nc.gpsimd.dma_start(xn[:, 2:4, :], xr[:, 2:4, :])
            emit_wqk_load(0, 0)
            nc.gpsimd.dma_start(xn[:, 4:6, :], xr[:, 4:6, :])
            emit_wqk_load(1, 0)
            nc.gpsimd.dma_start(xn[:, 6:8, :], xr[:, 6:8, :])
            bqk = const.tile([P, H], F32, tag="bqk")
            nc.gpsimd.dma_start(
                bqk[:], qkv_b[0 : H * P].rearrange("(t p) -> p t", p=P)
            )
            wv = const.tile([P, KT, D], BF16, tag="wv")
            nc.gpsimd.dma_start(
                wv[:], qkv_w[:, 2 * D : 3 * D].rearrange("(k p) d -> p k d", p=P)
            )
            for chunk in (1, 2):
                emit_wqk_load(0, chunk)
                emit_wqk_load(1, chunk)
            # bias rows (fp16 cast on load); broadcast across partitions on
            # the (otherwise idle) GpSimd engine. bv rides V (softmax rows
            # sum to 1 so adding bv to V adds bv to the normalized output);
            # bp is added during the proj eviction. This replaces the PE
            # bias-build matmul chain.
            bv_row = const.tile([1, D], BF16, tag="bv_row")
            nc.gpsimd.dma_start(
                bv_row[:], qkv_b[2 * D : 3 * D].rearrange("(o d) -> o d", o=1)
            )
            wp = const.tile([P, KT, D], BF16, tag="wp")
            nc.gpsimd.dma_start(wp[:], proj_w.rearrange("(k p) d -> p k d", p=P))
            bp_row = const.tile([1, D], BF16, tag="bp_row")
            nc.gpsimd.dma_start(bp_row[:], proj_b.rearrange("(o d) -> o d", o=1))
            bv_bc = const.tile([P, D], BF16, tag="bv_bc")
            nc.gpsimd.partition_broadcast(bv_bc[:], bv_row[:], channels=P)
            nc.gpsimd.partition_broadcast(bias_bc[:], bp_row[:], channels=P)
            # upper half of the last proj-weight chunk shifted to partitions
            # 0:64 (pairs with the odd head's pre-DMA normalize output in the
            # split pk=5 projection matmul)
            wp5hi = const.tile([HD, D], BF16, tag="wp5hi")
            nc.sync.dma_start(wp5hi[:], wp[HD:P, KT - 1, :])

            with tc.tile_pool(name="xt_ps", bufs=4, space="PSUM") as xtps_pool:
                for g in range(NT // 2):
                    for k in range(KT):
                        tp = xtps_pool.tile([P, 256], BF16, name="xtp")
                        for q in range(2):
                            nc.tensor.transpose(
                                tp[:, q * P : (q + 1) * P],
                                xn[:, 2 * g + q, k * P : (k + 1) * P],
                                ident[:],
                            )
                        if k < 3:
                            nc.vector.tensor_copy(
                                xT[k][:, g * 256 : (g + 1) * 256], tp[:]
                            )
                        else:
                            nc.scalar.activation(
                                xT[k][:, g * 256 : (g + 1) * 256], tp[:], AF.Copy
                            )
            xst_ctx.close()

        pexp_pool = ctx.enter_context(tc.tile_pool(name="pexp", bufs=28))
        sp_ctx = ExitStack()
        sp_pool = sp_ctx.enter_context(tc.tile_pool(name="s_ps", bufs=2, space="PSUM"))

        pe_store = {}  # (pair, jt) -> (peA, peB)

        def emit_scores(pr, jt):
            qt, kt = qkt[pr]
            sps = []
            for plo in (0, HD):
                sp = sp_pool.tile([P, SEQ], F32, tag="sp", name="sp")
                for lo in (0, 512):
                    nc.tensor.matmul(
                        sp[:, lo : lo + 512],
                        kt[plo : plo + HD, jt * P : (jt + 1) * P],
                        qt[plo : plo + HD, lo : lo + 512],
                        start=True, stop=True,
                        tile_position=(plo, 0),
                    )
                sps.append(sp)
            pes = []
            for sp in sps:
                pe = pexp_pool.tile([P, SEQ], BF16, tag="pe", name="pe")
                nc.scalar.activation(pe[:], sp[:], AF.Exp, bias=0.0, scale=float(SCALE))
                pes.append(pe)
            pe_store[(pr, jt)] = pes

        def emit_qk_tile(pr, a, qps_pool):
            # a=0 -> Q^T, a=1 -> K^T of pair pr
            qp = qps_pool.tile([P, SEQ], F32, tag="qp", name="qp")
            for lo in (0, 512):
                for k in range(KT):
                    nc.tensor.matmul(
                        qp[:, lo : lo + 512],
                        wqk_t[:, k, a, pr * P : (pr + 1) * P],
                        xT[k][:, lo : lo + 512],
                        start=(k == 0), stop=(k == KT - 1),
                    )
            qe = qkt[pr][a]
            with nc.allow_low_precision(reason="bf16 qkt"):
                nc.vector.tensor_scalar_add(
                    qe[:], qp[:], bqk[:, (a * NPAIR + pr) : (a * NPAIR + pr) + 1]
                )

        # ---- QK pairs 0-1 (enables early exp start on ScalarE) ----
        with tc.tile_pool(name="q_ps", bufs=2, space="PSUM") as qps_pool:
            for pr in (0, 1):
                for a in (0, 1):
                    emit_qk_tile(pr, a, qps_pool)

        # ---- V (interleaved with pair-0 scores+exp) ----
        def emit_v(nt):
            vp = vps_pool.tile([P, D], F32, name="vp")
            for lo, sz in ((0, 512), (512, 256)):
                for k in range(KT):
                    nc.tensor.matmul(
                        vp[:, lo : lo + sz],
                        xT[k][:, nt * P : (nt + 1) * P],
                        wv[:, k, lo : lo + sz],
                        start=(k == 0), stop=(k == KT - 1),
                    )
            v3 = vr[nt].rearrange("p (h c) -> p h c", h=H, c=65)
            nc.vector.tensor_copy(
                v3[:, :, 64:65], ones_h.rearrange("p (h o) -> p h o", h=H, o=1)
            )
            # on DVE (not ScalarE): keeps the activation engine free so the
            # pre-computed exps start as early as possible. The V bias is
            # added here (rows of softmax sum to 1, so V+bv shifts the
            # normalized output by exactly bv).
            with nc.allow_low_precision(reason="fp16 V"):
                nc.vector.tensor_add(
                    v3[:, :, 0:64],
                    vp.rearrange("p (h c) -> p h c", h=H, c=HD),
                    bv_bc.rearrange("p (h c) -> p h c", h=H, c=HD),
                )

        with tc.tile_pool(name="v_ps", bufs=2, space="PSUM") as vps_pool:
            for nt in range(NT):
                emit_v(nt)
                emit_scores(0, nt)

        # ---- QK pairs 2-5 (interleaved with pair-1 scores jt 0-3) ----
        with tc.tile_pool(name="q_ps2", bufs=2, space="PSUM") as qps_pool:
            units = [(pr, a) for pr in range(2, NPAIR) for a in (0, 1)]
            for i, (pr, a) in enumerate(units):
                emit_qk_tile(pr, a, qps_pool)
                if i < 6:
                    emit_scores(1, i)
            emit_scores(1, 6)
            emit_scores(1, 7)

        # ================= phase 2: attention =================
        def emit_av(pr, jt, oA, oB, evict=False):
            # evict=True (jt==NT-1, mid pairs): emit oA's eviction between
            # the two heads' matmul blocks so it overlaps oB's AV + the
            # boundary scores -- the next pair's first AV WAR-waits on it
            peA, peB = pe_store.pop((pr, jt))
            evs = []
            for h, o_ps, pe in ((2 * pr, oA, peA), (2 * pr + 1, oB, peB)):
                for lo in (0, 512):
                    nc.tensor.matmul(
                        o_ps[:, lo : lo + 512],
                        vr[jt][:, 65 * h : 65 * h + 65],
                        pe[:, lo : lo + 512],
                        start=(jt == 0), stop=(jt == NT - 1),
                    )
                if evict:
                    evs.append(emit_evict(o_ps))
            return tuple(evs) if evict else None

        def emit_evict(o_ps, on_act=False):
            osb = osb_pool.tile([65, SEQ], BF16, tag="osb", name="osb")
            with nc.allow_low_precision(reason="O' partial sums bf16"):
                if on_act:
                    nc.scalar.activation(osb[:], o_ps[:], AF.Copy)
                else:
                    nc.vector.tensor_copy(osb[:], o_ps[:])
            return osb

        def emit_normalize(pr, oAs, oBs, b_first=False, skip_b_dma=False):
            # per head: r (row 64) -> bcast over 64 partitions (GpSimd),
            # 1/r (DVE), O'*1/r (DVE). Odd head lands at partitions 64:128
            # of oT via DMA (DVE lanes can't shift partitions).
            # r row (partition 64) -> partition 0 via DMA; the broadcast
            # ucode only reads partition 0
            ret = None
            order = (1, 0) if b_first else (0, 1)
            for side in order:
                osrc = (oAs, oBs)[side]
                r0 = r0_pool.tile([1, SEQ], BF16, tag="r0", name="r0")
                nc.sync.dma_start(r0[:], osrc[64:65, :])
                rbc = rbc_pool.tile([HD, SEQ], BF16, tag="rbc", name="rbc")
                nc.gpsimd.partition_broadcast(rbc[:], r0[:], channels=HD)
                rec = rec_pool.tile([HD, SEQ], BF16, tag="rec", name="rec")
                with nc.allow_low_precision(reason="softmax denom recip"):
                    nc.vector.reciprocal(rec[:], rbc[:])
                with nc.allow_low_precision(reason="bf16 oT"):
                    if side == 0:
                        nc.vector.tensor_mul(
                            oT[pr][0:HD, :], osrc[0:HD, :], rec[:]
                        )
                    else:
                        ot_b = otb_pool.tile([HD, SEQ], BF16, tag="otb", name="otb")
                        nc.vector.tensor_mul(ot_b[:], osrc[0:HD, :], rec[:])
                        if not skip_b_dma:
                            nc.sync.dma_start(oT[pr][HD:P, :], ot_b[:])
                        ret = ot_b
            return ret

        def emit_normalize_last(pr, osbA, osbB):
            # pair-5 tail: both evictions run on ACT, so the DVE pipelines
            # recipA -> mulA with minimum latency; the even head's
            # oT[pr][0:64] is what gates the first pk=5 projection matmuls.
            r0A = r0_pool.tile([1, SEQ], BF16, tag="r0", name="r0A")
            nc.sync.dma_start(r0A[:], osbA[64:65, :])
            r0B = r0_pool.tile([1, SEQ], BF16, tag="r0", name="r0B")
            nc.sync.dma_start(r0B[:], osbB[64:65, :])
            rbcA = rbc_pool.tile([HD, SEQ], BF16, tag="rbc", name="rbcA")
            nc.gpsimd.partition_broadcast(rbcA[:], r0A[:], channels=HD)
            rbcB = rbc_pool.tile([HD, SEQ], BF16, tag="rbc", name="rbcB")
            nc.gpsimd.partition_broadcast(rbcB[:], r0B[:], channels=HD)
            recA = rec_pool.tile([HD, SEQ], BF16, tag="rec", name="recA")
            recB = rec_pool.tile([HD, SEQ], BF16, tag="rec", name="recB")
            with nc.allow_low_precision(reason="softmax denom recip"):
                nc.vector.reciprocal(recA[:], rbcA[:])
            with nc.allow_low_precision(reason="bf16 oT"):
                nc.vector.tensor_mul(oT[pr][0:HD, :], osbA[0:HD, :], recA[:])
            with nc.allow_low_precision(reason="softmax denom recip"):
                nc.vector.reciprocal(recB[:], rbcB[:])
            ot_b = otb_pool.tile([HD, SEQ], BF16, tag="otb", name="otb")
            with nc.allow_low_precision(reason="bf16 oT"):
                nc.vector.tensor_mul(ot_b[:], osbB[0:HD, :], recB[:])
            # also ship the odd head into oT[5]'s upper partitions: tiles
            # emitted late enough (the g-chunks) can then use the normal
            # single-matmul pk5 path (half the pk5 column cost)
            nc.sync.dma_start(oT[pr][HD:P, :], ot_b[:])
            return ot_b

        # ---- proj helpers (defined early: the first two proj front-chunks
        # are emitted inside the o-pool block, before it closes) ----
        pps = {}
        otb5_box = []
        ost_ctx = ExitStack()
        ost_pool = ost_ctx.enter_context(tc.tile_pool(name="ost", bufs=4))
        osth_pool = ost_ctx.enter_context(tc.tile_pool(name="osth", bufs=6))

        def emit_proj_chunks(pool, it, pks, start, stop, half=None,
                             split5=True):
            # half=None: full-width tile in a freed scores-PSUM slot (2
            # banks); half=0/1: a 384-col 1-bank piece -- fine-grained slot
            # recycling so late tiles never wait on a bias-add to free PSUM
            key = (it, half)
            if key not in pps:
                if half is None:
                    t = pool.tile([P, SEQ], F32, tag="sp", name="pp_sp")
                    pps[key] = t[:, 0:D]
                else:
                    pps[key] = pool.tile([P, 384], F32, tag="pph", name="pph")
            pp = pps[key]
            base = 0 if half is None else half * 384
            segs = ((0, 512), (512, 256)) if half is None else ((0, 384),)
            for lo, sz in segs:
                gl = base + lo
                for i, pk in enumerate(pks):
                    last = stop and i == len(pks) - 1
                    if pk == NPAIR - 1 and split5:
                        nc.tensor.matmul(
                            pp[:, lo : lo + sz],
                            oT[pk][0:HD, it * P : (it + 1) * P],
                            wp[0:HD, pk, gl : gl + sz],
                            start=(start and i == 0), stop=False,
                            tile_position=(0, 0),
                        )
                        nc.tensor.matmul(
                            pp[:, lo : lo + sz],
                            otb5_box[0][:, it * P : (it + 1) * P],
                            wp5hi[:, gl : gl + sz],
                            start=False, stop=last,
                            tile_position=(0, 0),
                        )
                    else:
                        nc.tensor.matmul(
                            pp[:, lo : lo + sz],
                            oT[pk][:, it * P : (it + 1) * P],
                            wp[:, pk, gl : gl + sz],
                            start=(start and i == 0), stop=last,
                        )

        # Single software-pipelined stream: the score/exp stream runs ~12
        # slots ahead of the AV stream (pair 0 + pair 1 jt0-3 banked during
        # phase 1), so ScalarE (the P2-bottleneck engine) never starves and
        # the PE never waits on exp.
        pending = {}  # pair -> (oAs, oBs)
        s_stream = [
            (p, jt) for p in range(2, NPAIR) for jt in range(NT)
        ]
        si = 0
        with tc.tile_pool(name="o_ps", bufs=2, space="PSUM") as ops_pool:
            for p in range(NPAIR):
                oA = ops_pool.tile([65, SEQ], F32, tag="o", name="oA")
                oB = ops_pool.tile([65, SEQ], F32, tag="o", name="oB")
                last_mid = p < NPAIR - 1
                for jt in range(NT):
                    # scores at 6-per-8 slots: the exp stream (2 exps per
                    # emission, ~2.08us) outpaces a 1/slot PE cadence
                    # (~1.71us); two scoreless slots per pair keep the
                    # sp-psum recycling ahead of the PE. They sit at jt 3/7
                    # so the jt==7 slot (which also carries the eviction
                    # interleave) never stalls on the ACT stream.
                    if si < len(s_stream) and jt not in (3, NT - 1):
                        emit_scores(*s_stream[si])
                        si += 1
                    evs = emit_av(
                        p, jt, oA, oB, evict=(last_mid and jt == NT - 1)
                    )
                    if jt == 0 and (p - 1) in pending:
                        emit_normalize(p - 1, *pending.pop(p - 1))
                if p == NPAIR - 1:
                    # A-side eviction on the (now idle) scalar engine, B on
                    # DVE: they run in parallel and the o-psum banks free
                    # for the first pp-pool proj pieces ~1us sooner
                    evA = emit_evict(oA, on_act=True)
                    evB = emit_evict(oB)
                else:
                    pending[p] = evs

            # ================= phase 3: proj =================
            # last pair's normalization overlaps the first proj chunks; its
            # oT contribution (pk=5) is ordered last in every accumulation
            # and, for the odd head, is taken straight from the pre-DMA
            # normalize output (ot_b at partitions 0:64) as a second K=64
            # matmul so the partition-shifting DMA never gates the proj.
            # f0/f1 ride freed scores-PSUM slots and are emitted before the
            # o-pool closes: they keep the PE fed across the pool-close
            # barrier (which waits on the pair-5 evictions)
            emit_proj_chunks(sp_pool, 0, range(5), start=True, stop=False)
            emit_proj_chunks(sp_pool, 1, range(5), start=True, stop=False)

        # pair-5 normalize AFTER the o-pool closes: its DVE ops would
        # otherwise sit ahead of the pool-release drain on the DVE queue and
        # delay the first pp-pool proj pieces by the whole chain
        otb5_box.append(emit_normalize_last(NPAIR - 1, evA, evB))

        def emit_proj_out(it, q, half=None):
            pp = pps.pop((it, half))
            if half is None:
                ot = ost_pool.tile([P, D], F32, tag="ot", name="ot")
                for (lo, sz), sq in zip(((0, 384), (384, 384)),
                                        (nc.sync, nc.scalar)):
                    nc.vector.tensor_add(
                        ot[:, lo : lo + sz], pp[:, lo : lo + sz],
                        bias_bc[:, lo : lo + sz],
                    )
                    sq.dma_start(
                        out[it * P : (it + 1) * P, lo : lo + sz],
                        ot[:, lo : lo + sz],
                    )
            else:
                base = half * 384
                ot = osth_pool.tile([P, 384], F32, tag="oth", name="oth")
                nc.vector.tensor_add(ot[:], pp[:], bias_bc[:, base : base + 384])
                q.dma_start(
                    out[it * P : (it + 1) * P, base : base + 384], ot[:]
                )

        with tc.tile_pool(name="p_ps", bufs=4, space="PSUM") as pp_pool:
            qs = [nc.sync, nc.scalar]
            # tiles 0/1 (full, sp slots, emitted in-pool) + tiles 2/3 as
            # 1-bank half-pieces cover the pair-5 normalize chain; their pk5
            # tails run once it lands. Tiles 4-7 are emitted after the chain
            # completes, so they run as one g-chunk with pk5 FIRST (start)
            # and pk0-4 behind (stop) -- no pk5 stall on the PE queue head.
            # Half-pieces recycle PSUM at 1-bank granularity so the PE never
            # waits on a bias-add.
            seq = [
                ("f", 2, 0), ("f", 2, 1), ("f", 3, 0), ("f", 3, 1),
                ("t", 0, None), ("t", 1, None),
                ("g", 4, None), ("g", 5, None),
                ("t", 2, 0), ("t", 2, 1), ("t", 3, 0), ("t", 3, 1),
                ("g", 6, 0), ("g", 6, 1), ("g", 7, 0), ("g", 7, 1),
            ]
            for n, (kind, it, half) in enumerate(seq):
                pool = sp_pool if half is None else pp_pool
                if kind == "f":
                    # scheduler hint: these pieces are pure gap-fillers for
                    # the pair-5 normalize window -- keep them ahead of the
                    # pk5-dependent chunks on the PE queue
                    with tc.high_priority():
                        emit_proj_chunks(pool, it, range(5), start=True,
                                         stop=False, half=half)
                elif False:
                    pass
                else:
                    pks = [5, 0, 1, 2, 3, 4] if kind == "g" else [5]
                    emit_proj_chunks(pool, it, pks, start=(kind == "g"),
                                     stop=True, half=half,
                                     split5=(kind != "g"))
                    emit_proj_out(it, qs[n % 2], half=half)
        sp_ctx.close()
        ost_ctx.close()

    return nc


_NC_CACHE = {}


def _get_nc():
    if "nc" not in _NC_CACHE:
        nc = bacc.Bacc("TRN2", target_bir_lowering=False, debug=False)
        build_mhsa(nc)
        nc.compile()
        _NC_CACHE["nc"] = nc
    return _NC_CACHE["nc"]


def kernel(x, qkv_w, qkv_b, proj_w, proj_b, _trace=False, _trace_kwargs=None):
    x = np.ascontiguousarray(np.asarray(x, dtype=np.float32))
    B = x.shape[0]
    assert x.shape == (B, SEQ, D)
    nc = _get_nc()
    shared = {
        "qkv_w": np.ascontiguousarray(np.asarray(qkv_w, np.float32)),
        "qkv_b": np.ascontiguousarray(np.asarray(qkv_b, np.float32)),
        "proj_w": np.ascontiguousarray(np.asarray(proj_w, np.float32)),
        "proj_b": np.ascontiguousarray(np.asarray(proj_b, np.float32)),
    }
    in_maps = [{"x": x[b], **shared} for b in range(B)]
    res = run_bass_kernel_spmd(
        nc, in_maps, list(range(B)), trace=_trace, **(_trace_kwargs or {})
    )
    out = np.stack([res.results[b]["out"] for b in range(B)])
    if _trace:
        return out, res
    return out

